# revision 55
# baseline (speedup 1.0000x reference)
"""Trainium2 Bass kernel for a dense transformer encoder layer.

Full (unsharded) contract: kernel(**inputs) -> np.ndarray.

Model: B=4, S=2048, D=768, H=12 heads of 64, FFN 3072, two LayerNorms,
softmax attention (no mask).

Sharding: 8 cores, one (batch, query-half) pair per core — batch is data
parallel, each batch's 2048 query rows split across 2 cores. Each core
recomputes K/V for the full 2048-row sequence of its batch, so there are
no collectives and one SPMD program serves all cores. Per-core inputs are
row-permuted so the core's own 1024 query rows always come first.

Numerics/layout choices:
- All weights are transposed (W1 also pre-scaled by the LN1 gain, b1
  shifted by the LN1 bias) on the HOST and shipped as bf16 (W1 as
  scaled fp8 for DoubleRow matmuls): zero device weight transposes,
  half the weight DMA bytes.
- x is shipped twice, both bf16: x^T (matmul operand) and natural rows
  (residual path).
- Matmuls run in bf16 with fp32 PSUM accumulation; FFN1 runs fp8
  DoubleRow (2 contraction chunks per pass, 2x PE rate). LayerNorm
  statistics in fp32. Softmax skips the row-max subtraction (scores are
  far from fp32 overflow); denominators come from a ones-column appended
  to V and normalization (folded into the context-block PSUM eviction)
  is deferred one head so its reciprocal chain never stalls the PE.
- Projections and attention are a single merged phase: after a prologue
  (all Q, K chunk 0, all V), the remaining K feature-chunks are emitted
  as PE filler work between attention heads, soaking up the PE slack
  while the Act engine paces the softmax exps.
- FFN computes all 24 ReLU'd intermediate row-chunks first, then one
  PSUM-resident accumulation pass per output row-chunk, fusing the
  second residual + LN2 + store into that pass's tail.
"""

from contextlib import ExitStack

import numpy as np

import concourse.bass as bass
import concourse.tile as tile
from concourse import mybir
from concourse.bass_utils import run_bass_kernel_spmd
from concourse.masks import make_identity
from concourse.vector_clock import ScopedClock

F32 = mybir.dt.float32
F32R = mybir.dt.float32r
BF = mybir.dt.bfloat16
F8 = mybir.dt.float8e4
AF = mybir.ActivationFunctionType
ALU = mybir.AluOpType
DR = mybir.MatmulPerfMode.DoubleRow

# fp8(e4m3) double-row FFN: weights/activations pre-scaled by W8SCALE on
# the host so their magnitudes sit in e4m3's normal range; the inverse
# scale is folded into the ReLU activation / FFN2 eviction.
FP8_FFN = "w1"            # 'full' | 'w1' | 'ffn2' | 'off'
FP8_W1 = FP8_FFN in ("full", "w1")
FP8_W2 = FP8_FFN in ("full", "ffn2")
W8SCALE = 64.0

P = 128
B, S, D, H, E, DFF = 4, 2048, 768, 12, 64, 3072
SQ = S // 2            # query rows per core
DC = D // P            # 6 d-chunks
SC = SQ // P           # 8 query-row chunks
TC = S // P            # 16 key-row chunks
FC = DFF // P          # 24 ffn-row chunks
WBLK = 512             # attention query-block width
EPS = 1e-5

_MAX_WAITS = 1


class _PatchedTileContext(tile.TileContext):
    """This container's walrus rejects instructions with >1 sync wait.

    Hoist all but one wait of each committed instruction onto same-engine
    no-ops emitted just before it (sequential waits on one queue are
    equivalent to a combined wait), and split the teardown drain the same
    way.
    """

    def _split_waits(self, inst) -> None:
        si = inst.sync_info
        if si is None or not si.on_wait or len(si.on_wait) <= _MAX_WAITS:
            return
        if inst.engine == mybir.EngineType.Unassigned:
            return
        waits = list(si.on_wait)
        keep = waits[-_MAX_WAITS:]
        hoist = waits[:-_MAX_WAITS]
        for i in range(0, len(hoist), _MAX_WAITS):
            nop = mybir.InstNoOp(
                name=f"I-waitsplit-{self.nc.next_id()}",
                engine=inst.engine,
                bass_nofuse=True,
                sync_info=mybir.SyncInfo(
                    on_wait=hoist[i : i + _MAX_WAITS], on_update=[]
                ),
            )
            self._add_instruction(nop)
        inst.sync_info = mybir.SyncInfo(on_wait=keep, on_update=si.on_update)

    def _commit_instruction(self, inst, lazy_reg_writes: bool = True):
        if isinstance(inst, mybir.Instruction):
            self._split_waits(inst)
        return super()._commit_instruction(inst, lazy_reg_writes)

    def _drain_and_barrier(self, tick_clock, wait_clock):
        probe = self.nc.sync.nop(nofuse=True, hint="drain_wait_split")
        wait_clock.add_sem_waits(
            probe.ins, ScopedClock({None: tick_clock.global_clock})
        )
        self._split_waits(probe.ins)
        self.nc.sync.drain()

        self.nc.all_engine_barrier()
        assert self.sems is not None
        popped = self.nc._tile_sem_poison_stack.pop()
        assert popped is self._sem_poison
        self.nc.clear_and_free_semaphores(list(self.sems.allocated().values()))
        self.nc.all_engine_barrier()


def _ln_stats(nc, sp, src, eps_t):
    """Return (mean_var, rstd) tiles for rows of src [128, D]."""
    stats = sp.tile([P, 3, 6], F32, tag="ln_stats")
    for g3 in range(3):
        nc.vector.bn_stats(out=stats[:, g3, :], in_=src[:, g3 * 256 : (g3 + 1) * 256])
    mv = sp.tile([P, 2], F32, tag="ln_mv")
    nc.vector.bn_aggr(out=mv[:], in_=stats[:])
    std = sp.tile([P, 1], F32, tag="ln_std")
    nc.scalar.activation(out=std[:], in_=mv[:, 1:2], func=AF.Sqrt, bias=eps_t[:])
    rstd = sp.tile([P, 1], F32, tag="ln_rstd")
    nc.vector.reciprocal(out=rstd[:], in_=std[:])
    return mv, rstd


STOP_AFTER = None


def build_nc(stop_after=None):
    nc = bass.Bass("TRN2", target_bir_lowering=False, debug=False, num_devices=8)

    xt = nc.dram_tensor("xt", [D, S], BF, kind="ExternalInput").ap()
    xq = nc.dram_tensor("xq", [SQ, D], BF, kind="ExternalInput").ap()
    wqt = nc.dram_tensor("wqt", [D, D], BF, kind="ExternalInput").ap()
    wkt = nc.dram_tensor("wkt", [D, D], BF, kind="ExternalInput").ap()
    wvt = nc.dram_tensor("wvt", [D, D], BF, kind="ExternalInput").ap()
    wot = nc.dram_tensor("wot", [D, D], BF, kind="ExternalInput").ap()
    if FP8_W1:
        w1t = nc.dram_tensor("w1t", [3 * P, 2 * DFF], F8, kind="ExternalInput").ap()
    else:
        w1t = nc.dram_tensor("w1t", [D, DFF], BF, kind="ExternalInput").ap()
    if FP8_W2:
        w2t = nc.dram_tensor("w2t", [12 * P, 2 * D], F8, kind="ExternalInput").ap()
    else:
        w2t = nc.dram_tensor("w2t", [DFF, D], BF, kind="ExternalInput").ap()
    b1c = nc.dram_tensor("b1c", [P, FC], F32, kind="ExternalInput").ap()
    g1v = nc.dram_tensor("g1v", [D], F32, kind="ExternalInput").ap()
    g2v = nc.dram_tensor("g2v", [D], F32, kind="ExternalInput").ap()
    be2v = nc.dram_tensor("be2v", [D], F32, kind="ExternalInput").ap()
    c2v = nc.dram_tensor("c2v", [D], F32, kind="ExternalInput").ap()
    out = nc.dram_tensor("out", [SQ, D], F32, kind="ExternalOutput").ap()

    with _PatchedTileContext(nc) as tc, ExitStack() as top:
        const = top.enter_context(tc.tile_pool(name="const", bufs=1))

        # ---- constants -------------------------------------------------
        eps_t = const.tile([P, 1], F32)
        nc.vector.memset(eps_t[:], EPS)
        ones_f = const.tile([P, E], F32)
        nc.vector.memset(ones_f[:], 1.0)
        ones_r = const.tile([P, E], F32R)            # rows of ones
        nc.any.tensor_copy(ones_r[:], ones_f[:])
        ones_fr = const.tile([1, P], F32)
        nc.vector.memset(ones_fr[:], 1.0)
        ones_row = const.tile([1, P], F32R)          # broadcast lhsT
        nc.any.tensor_copy(ones_row[:], ones_fr[:])
        ident = const.tile([P, P], F32)
        make_identity(nc, ident)
        identb = const.tile([P, P], BF)
        nc.any.tensor_copy(identb[:], ident[:])

        b1_cols = const.tile([P, FC], F32)
        nc.gpsimd.dma_start(out=b1_cols[:], in_=b1c[:, :])

        # per-feature vectors broadcast to all partitions via PE outer
        # product: bcast = ones_row.T @ vec
        vec_bc = {}
        with tc.tile_pool(name="bc_psum", bufs=2, space="PSUM") as bc_psum, \
             tc.tile_pool(name="vecstage", bufs=2) as vst:
            for name, ap in (("g1", g1v), ("g2", g2v), ("be2", be2v),
                             ("c2", c2v)):
                raw = vst.tile([1, D], F32, tag="vecraw")
                nc.gpsimd.dma_start(out=raw[:], in_=ap[None, :])
                rr = vst.tile([1, D], F32R, tag="vecr")
                nc.any.tensor_copy(rr[:], raw[:])
                bc = const.tile([P, D], F32, tag=f"bc_{name}")
                for of in range(2):
                    ps = bc_psum.tile([P, 384], F32, tag="bcps")
                    nc.tensor.matmul(ps[:], ones_row[:], rr[:, bass.ts(of, 384)])
                    nc.any.tensor_copy(bc[:, bass.ts(of, 384)], ps[:])
                vec_bc[name] = bc

        # ---- long-lived activation tensors -----------------------------
        # QT/KT in fp8: scores lose ~0.5% accuracy (washed out by softmax
        # normalization + LN) and SBUF drops 18KB, making room to run
        # attention concurrently with the projections.
        kqv_pool = tc.alloc_tile_pool(name="kqv", bufs=1)
        KT = [kqv_pool.tile([P, S], BF, tag=f"kt{i}", name=f"kt{i}") for i in range(DC)]
        QT = [kqv_pool.tile([P, SQ], BF, tag=f"qt{i}", name=f"qt{i}") for i in range(DC)]
        VP = [kqv_pool.tile([P, H * (E + 1)], BF, tag=f"vp{i}", name=f"vp{i}") for i in range(TC)]

        # ---- phase-1 operand pools; xw_b (wq/wv) dies after the
        # prologue, xw_a (xt/wk) after the last K filler mid-attention.
        # x^T in half-tiles: Q only reads the low half (its own query
        # rows), so the first projections start after ~60% of the loads
        xw_a = tc.alloc_tile_pool(name="xwa", bufs=1)
        xlo = [xw_a.tile([P, SQ], BF, tag=f"xl{d}", name=f"xlo{d}") for d in range(DC)]
        xhi = [xw_a.tile([P, SQ], BF, tag=f"xh{d}", name=f"xhi{d}") for d in range(DC)]
        wktt = [xw_a.tile([P, D], BF, tag=f"wk{d}", name=f"wktt{d}") for d in range(DC)]
        xw_b = tc.alloc_tile_pool(name="xwb", bufs=1)
        wqtt = [xw_b.tile([P, D], BF, tag=f"wq{d}", name=f"wqtt{d}") for d in range(DC)]
        wvtt = [xw_b.tile([P, D], BF, tag=f"wv{d}", name=f"wvtt{d}") for d in range(DC)]
        for dc in range(3):
            nc.sync.dma_start(out=xlo[dc][:], in_=xt[bass.ts(dc, P), 0:SQ])
            nc.sync.dma_start(out=wqtt[dc][:], in_=wqt[bass.ts(dc, P), :])
            nc.gpsimd.dma_start(out=xlo[3 + dc][:], in_=xt[bass.ts(3 + dc, P), 0:SQ])
            nc.gpsimd.dma_start(out=wqtt[3 + dc][:], in_=wqt[bass.ts(3 + dc, P), :])
        for dc in range(3):
            nc.sync.dma_start(out=wktt[dc][:], in_=wkt[bass.ts(dc, P), :])
            nc.gpsimd.dma_start(out=wktt[3 + dc][:], in_=wkt[bass.ts(3 + dc, P), :])
        for dc in range(3):
            nc.sync.dma_start(out=xhi[dc][:], in_=xt[bass.ts(dc, P), SQ:S])
            nc.gpsimd.dma_start(out=xhi[3 + dc][:], in_=xt[bass.ts(3 + dc, P), SQ:S])
        for dc in range(DC):
            nc.gpsimd.dma_start(out=wvtt[dc][:], in_=wvt[bass.ts(dc, P), :])

        def xpart(st):
            """(half-tile list, local index) for a 512-wide column block."""
            return (xlo, st) if st < SQ // WBLK else (xhi, st - SQ // WBLK)

        _ORDER = {None: 4, "proj": 2, "attn": 2, "wo": 3}
        _lvl = _ORDER[stop_after]

        # ===============================================================
        # Merged phase 1+2: Q/K/V projections interleaved with attention.
        # Prologue computes Q/K for feature-chunk 0 and V for the full
        # sequence, so heads 0-1 start while the remaining Q/K chunks are
        # emitted as PE filler work between attention heads (the Act
        # engine paces attention; fillers soak up PE slack).  At head 8
        # the projection operand pool is swapped for the Wo/xq/W1
        # prefetch pool.
        # ===============================================================
        w1_pool = tc.alloc_tile_pool(name="w1p", bufs=1, side="right")
        if FP8_W1:
            w1f = [w1_pool.tile([P, 2, DFF], F8, tag=f"w1f{d}", name=f"w1f{d}") for d in range(3)]
        else:
            w1f = [w1_pool.tile([P, DFF], BF, tag=f"w1f{d}", name=f"w1f{d}") for d in range(DC)]
        wx_pool = tc.alloc_tile_pool(name="wx", bufs=1, side="right")
        wott = [wx_pool.tile([P, D], BF, tag=f"wo{d}", name=f"wott{d}") for d in range(DC)]
        xqt = [wx_pool.tile([P, D], BF, tag=f"xq{i}", name=f"xqt{i}") for i in range(SC)]
        concat_pool = tc.alloc_tile_pool(name="concat", bufs=1, side="right")
        concatT = [concat_pool.tile([P, SQ], BF, tag=f"cc{i}", name=f"cc{i}") for i in range(DC)]

        with ExitStack() as ph:
            expp = ph.enter_context(tc.tile_pool(name="expp", bufs=13, side="right"))
            rcp = ph.enter_context(tc.tile_pool(name="rcp", bufs=1, side="right"))
            tmp64 = ph.enter_context(tc.tile_pool(name="tmp64", bufs=1, side="right"))
            fil = ph.enter_context(tc.tile_pool(name="fil", bufs=2, space="PSUM"))
            sps = ph.enter_context(tc.tile_pool(name="sps", bufs=2, space="PSUM"))
            cps = ph.enter_context(tc.tile_pool(name="cps", bufs=1, space="PSUM"))

            def q_group(fc, st):
                ps = fil.tile([P, WBLK], F32, tag="fil")
                for dc in range(DC):
                    nc.tensor.matmul(
                        ps[:],
                        wqtt[dc][:, bass.ts(fc, P)],
                        xlo[dc][:, bass.ts(st, WBLK)],
                        start=(dc == 0), stop=(dc == DC - 1),
                    )
                with nc.allow_low_precision(reason="fp8 scores operands"):
                    nc.any.tensor_copy(QT[fc][:, bass.ts(st, WBLK)], ps[:])

            def k_group(fc, st):
                ps = fil.tile([P, WBLK], F32, tag="fil")
                xh, sl = xpart(st)
                for dc in range(DC):
                    nc.tensor.matmul(
                        ps[:],
                        wktt[dc][:, bass.ts(fc, P)],
                        xh[dc][:, bass.ts(sl, WBLK)],
                        start=(dc == 0), stop=(dc == DC - 1),
                    )
                with nc.allow_low_precision(reason="fp8 scores operands"):
                    nc.any.tensor_copy(KT[fc][:, bass.ts(st, WBLK)], ps[:])

            def v_group(r):
                vtile = VP[r]
                v3 = vtile.rearrange("p (h q) -> p h q", q=E + 1)
                for of in range(2):
                    ps = fil.tile([P, WBLK], F32, tag="fil")
                    vh, vl = (xlo, r) if r < SC else (xhi, r - SC)
                    for dc in range(DC):
                        nc.tensor.matmul(
                            ps[:, 0:384],
                            vh[dc][:, bass.ts(vl, P)],
                            wvtt[dc][:, bass.ts(of, 384)],
                            start=(dc == 0), stop=(dc == DC - 1),
                        )
                    nc.any.tensor_copy(
                        v3[:, bass.ds(6 * of, 6), 0:E],
                        ps[:, 0:384].rearrange("p (h e) -> p h e", e=E),
                    )
                nc.any.tensor_copy(
                    v3[:, :, E : E + 1], ones_f[:, 0:H].unsqueeze(2)
                )

            fillers = [
                (fc, st) for fc in range(1, DC) for st in range(S // WBLK)
            ]
            done_fc = [1]  # K chunks 0..done_fc[0]-1 fully emitted

            def emit_fillers(n):
                for _ in range(n):
                    if not fillers:
                        if xw_state[0] == 0:
                            xw_state[0] = 1
                            xw_a.release()
                        return
                    fc, st = fillers.pop(0)
                    k_group(fc, st)
                    if not fillers or fillers[0][0] != fc:
                        done_fc[0] = fc + 1

            def need_fc(fc):
                while done_fc[0] <= fc and fillers:
                    emit_fillers(1)

            xw_state = [0]

            def scores_exp(h):
                hp, off = divmod(h, 2)
                off *= E
                expts = []
                for t in range(TC):
                    ps_s = sps.tile([P, 2 * WBLK], F32, tag="ps_s")
                    for sh in range(2):
                        nc.tensor.matmul(
                            ps_s[:, bass.ts(sh, WBLK)],
                            KT[hp][off : off + E, bass.ts(t, P)],
                            QT[hp][off : off + E, bass.ts(sh, WBLK)],
                        )
                    ex = expp.tile([P, 2 * WBLK], BF, tag="expt")
                    nc.scalar.activation(
                        out=ex[:], in_=ps_s[:], func=AF.Exp, scale=0.125
                    )
                    expts.append(ex)
                return expts

            def ctx_accum(h, expts):
                ps_cs = [cps.tile([E + 1, WBLK], F32, tag=f"ps_c{i}", name=f"psc{h}_{i}") for i in range(2)]
                for t in range(TC):
                    for sh in range(2):
                        nc.tensor.matmul(
                            ps_cs[sh][:],
                            VP[t][:, h * (E + 1) : (h + 1) * (E + 1)],
                            expts[t][:, bass.ds(sh * WBLK, WBLK)],
                            start=(t == 0), stop=(t == TC - 1),
                        )
                return ps_cs

            def norm_head(h, ps_cs):
                hp, off = divmod(h, 2)
                off *= E
                for sh in range(2):
                    sblk = bass.ds(sh * WBLK, WBLK)
                    ps_c = ps_cs[sh]
                    rt = rcp.tile([P, WBLK], F32R, tag="recip")
                    with nc.allow_low_precision(
                        reason="fp32r-rounded softmax denominators"
                    ):
                        nc.vector.reciprocal(rt[E : E + 1, :], ps_c[E : E + 1, :])
                    ps_b = fil.tile([P, WBLK], F32, tag="fil")
                    nc.tensor.matmul(
                        ps_b[0:E, :], ones_r[E : E + 1, :], rt[E : E + 1, :]
                    )
                    rbc = tmp64.tile([E, WBLK], BF, tag="rbc")
                    nc.any.tensor_copy(rbc[:], ps_b[0:E, :])
                    if off == 0:
                        nc.any.tensor_mul(
                            concatT[hp][0:E, sblk], ps_c[0:E, :], rbc[:]
                        )
                    else:
                        tt = tmp64.tile([E, WBLK], BF, tag="ctmp")
                        nc.any.tensor_mul(tt[:], ps_c[0:E, :], rbc[:])
                        nc.sync.dma_start(
                            out=concatT[hp][E : 2 * E, sblk], in_=tt[:]
                        )

            # ---- prologue: all Q, K chunk 0, V for the full sequence ----
            for fc in range(DC):
                for st in range(SQ // WBLK):
                    q_group(fc, st)
            for st in range(S // WBLK):
                k_group(0, st)
            for r in range(TC):
                v_group(r)
            xw_b.release()

            # prefetch DMAs queue behind the projection loads
            for dc in range(DC):
                nc.sync.dma_start(out=wott[dc][:], in_=wot[bass.ts(dc, P), :])
            for r in range(SC):
                nc.sync.dma_start(out=xqt[r][:], in_=xq[bass.ts(r, P), :])
            if FP8_W1:
                for j in range(3):
                    nc.gpsimd.dma_start(
                        out=w1f[j][:].rearrange("p i f -> p (i f)"),
                        in_=w1t[bass.ts(j, P), :],
                    )
            else:
                for dc in range(DC):
                    nc.gpsimd.dma_start(out=w1f[dc][:], in_=w1t[bass.ts(dc, P), :])

            # ---- head slots; remaining K chunks emitted as PE fillers,
            # normalization deferred one head so its recip chain never
            # stalls the PE
            pending = None
            for h in range(H):
                need_fc(h // 2)
                ex = scores_exp(h)
                emit_fillers(2)
                if pending is not None:
                    norm_head(*pending)
                pending = (h, ctx_accum(h, ex))
            norm_head(*pending)
            emit_fillers(len(fillers) + 1)

        if _lvl < 3:
            concat_pool.release()
            wx_pool.release()
            w1_pool.release()
            kqv_pool.release()
            return nc

        # ===============================================================
        # Phase 3: Wo + residual + LN1 (affine folded into W1/b1) ->
        # hbf (bf16 normalized rows) and hT.  Prefetch W2.
        # ===============================================================
        kqv_pool.release()
        sp = tc.alloc_tile_pool(name="scratch", bufs=3)
        hht_pool = tc.alloc_tile_pool(name="hht", bufs=1)
        hbf = [hht_pool.tile([P, D], BF, tag=f"h{i}", name=f"h{i}") for i in range(SC)]
        if FP8_W1:
            hT_all = hht_pool.tile([P, DC, SQ], F8, tag="htall", name="htall")
            hT = [hT_all[:, dc] for dc in range(DC)]
        else:
            hT = [hht_pool.tile([P, SQ], BF, tag=f"ht{i}", name=f"ht{i}") for i in range(DC)]
        w2_pool = tc.alloc_tile_pool(name="w2p", bufs=1)
        if FP8_W2:
            w2f = [w2_pool.tile([P, 2, D], F8, tag=f"w2f{i}", name=f"w2f{i}") for i in range(12)]
            for j in range(12):
                nc.gpsimd.dma_start(
                    out=w2f[j][:].rearrange("p i f -> p (i f)"),
                    in_=w2t[bass.ts(j, P), :],
                )
        else:
            w2f = [w2_pool.tile([P, D], BF, tag=f"w2f{i}", name=f"w2f{i}") for i in range(FC)]
            for ic in range(FC):
                nc.gpsimd.dma_start(out=w2f[ic][:], in_=w2t[bass.ts(ic, P), :])

        with ExitStack() as ph:
            tps = ph.enter_context(tc.tile_pool(name="tps4", bufs=4, space="PSUM"))
            ops = ph.enter_context(tc.tile_pool(name="ops4", bufs=4, space="PSUM"))

            for r in range(SC):
                res1 = sp.tile([P, D], F32, tag="res1")
                for of in range(2):
                    ps = ops.tile([P, 384], F32, tag="wops")
                    for cc in range(DC):
                        nc.tensor.matmul(
                            ps[:],
                            concatT[cc][:, bass.ts(r, P)],
                            wott[cc][:, bass.ts(of, 384)],
                            start=(cc == 0), stop=(cc == DC - 1),
                        )
                    nc.vector.tensor_add(
                        res1[:, bass.ts(of, 384)], ps[:], xqt[r][:, bass.ts(of, 384)]
                    )
                mv, rstd = _ln_stats(nc, sp, res1, eps_t)
                nc.vector.tensor_scalar(
                    out=hbf[r][:], in0=res1[:], scalar1=mv[:, 0:1], scalar2=rstd[:],
                    op0=ALU.subtract, op1=ALU.mult,
                )
                for dc in range(DC):
                    ps = tps.tile([P, P], BF, tag="tp4")
                    nc.tensor.transpose(ps[:], hbf[r][:, bass.ts(dc, P)], identb[:])
                    with nc.allow_low_precision(reason="fp8 ffn operand"):
                        nc.any.tensor_copy(hT[dc][:, bass.ts(r, P)], ps[:])

        concat_pool.release()
        wx_pool.release()

        if _lvl < 4:
            w2_pool.release()
            hht_pool.release()
            sp.release()
            w1_pool.release()
            return nc

        # ===============================================================
        # Phase 4a: FFN1 — all 24 ReLU'd intermediate row-chunks
        # ===============================================================
        relup = tc.alloc_tile_pool(name="relu", bufs=1)
        prer_pool = tc.alloc_tile_pool(name="prer", bufs=1)
        prer = [prer_pool.tile([P, D], F32, tag=f"pr{i}", name=f"pr{i}") for i in range(SC)]
        if FP8_W2:
            relu_all = relup.tile([P, FC, SQ], F8, tag="rla", name="rla")
            relu1T = [relu_all[:, i] for i in range(FC)]
        else:
            relu1T = [relup.tile([P, SQ], BF, tag=f"rl{i}", name=f"rl{i}") for i in range(FC)]
        with ExitStack() as ph:
            f1ps = ph.enter_context(tc.tile_pool(name="f1ps", bufs=3, space="PSUM"))
            for gic in range(FC):
                for st in range(SQ // WBLK):
                    ps = f1ps.tile([P, WBLK], F32, tag="f1")
                    if FP8_W1:
                        for j in range(3):
                            nc.tensor.matmul(
                                ps[:],
                                w1f[j][:, :, bass.ts(gic, P)],
                                hT_all[:, bass.ds(2 * j, 2), bass.ts(st, WBLK)],
                                start=(j == 0), stop=(j == 2),
                                perf_mode=DR,
                            )
                    else:
                        for dc in range(DC):
                            nc.tensor.matmul(
                                ps[:],
                                w1f[dc][:, bass.ts(gic, P)],
                                hT[dc][:, bass.ts(st, WBLK)],
                                start=(dc == 0), stop=(dc == DC - 1),
                            )
                    # bias+ReLU as one op on alternating engines; values
                    # stay at the x64 weight scale (unscaled at FFN2 evict)
                    with nc.allow_low_precision(reason="fp8 ffn operand"):
                        if st == 0:
                            nc.scalar.activation(
                                out=relu1T[gic][:, bass.ts(st, WBLK)], in_=ps[:],
                                func=AF.Relu, bias=b1_cols[:, gic : gic + 1],
                            )
                        else:
                            nc.vector.tensor_scalar(
                                out=relu1T[gic][:, bass.ts(st, WBLK)], in0=ps[:],
                                scalar1=b1_cols[:, gic : gic + 1], scalar2=0.0,
                                op0=ALU.add, op1=ALU.max,
                            )
        w1_pool.release()

        # ===============================================================
        # Phase 4b: FFN2 per row-chunk with PSUM-held accumulation,
        # fused residual2 + LN2 + store
        # ===============================================================
        with ExitStack() as ph:
            f2ps = ph.enter_context(tc.tile_pool(name="f2ps", bufs=4, space="PSUM"))
            for r in range(SC):
                pss = []
                for of in range(2):
                    ps = f2ps.tile([P, 384], F32, tag="f2")
                    if FP8_W2:
                        for j in range(12):
                            nc.tensor.matmul(
                                ps[:],
                                relu_all[:, bass.ds(2 * j, 2), bass.ts(r, P)],
                                w2f[j][:, :, bass.ts(of, 384)],
                                start=(j == 0), stop=(j == 11),
                                perf_mode=DR,
                            )
                    else:
                        for ic in range(FC):
                            nc.tensor.matmul(
                                ps[:],
                                relu1T[ic][:, bass.ts(r, P)],
                                w2f[ic][:, bass.ts(of, 384)],
                                start=(ic == 0), stop=(ic == FC - 1),
                            )
                    pss.append(ps)
                # residual2 = hbf*g1 + c2 + ffn; LN2 normalize on Act
                res2 = prer[r]
                nc.vector.tensor_mul(res2[:], hbf[r][:], vec_bc["g1"][:])
                nc.vector.tensor_add(res2[:], res2[:], vec_bc["c2"][:])
                unscale = 1.0
                if FP8_W1:
                    unscale /= W8SCALE
                if FP8_W2:
                    unscale /= W8SCALE
                for of in range(2):
                    if unscale != 1.0:
                        nc.vector.scalar_tensor_tensor(
                            out=res2[:, bass.ts(of, 384)], in0=pss[of][:],
                            scalar=unscale,
                            in1=res2[:, bass.ts(of, 384)],
                            op0=ALU.mult, op1=ALU.add,
                        )
                    else:
                        nc.vector.tensor_add(
                            res2[:, bass.ts(of, 384)], res2[:, bass.ts(of, 384)],
                            pss[of][:],
                        )
                mv, rstd = _ln_stats(nc, sp, res2, eps_t)
                nmr = sp.tile([P, 1], F32, tag="nmr")
                nc.vector.tensor_scalar(
                    out=nmr[:], in0=mv[:, 0:1], scalar1=rstd[:],
                    scalar2=-1.0, op0=ALU.mult, op1=ALU.mult,
                )
                o = sp.tile([P, D], F32, tag="otile")
                nc.scalar.activation(
                    out=o[:], in_=res2[:], func=AF.Identity,
                    scale=rstd[:], bias=nmr[:],
                )
                nc.vector.tensor_mul(o[:], o[:], vec_bc["g2"][:])
                nc.vector.tensor_add(o[:], o[:], vec_bc["be2"][:])
                nc.sync.dma_start(out=out[bass.ts(r, P), :], in_=o[:])

        prer_pool.release()
        relup.release()
        w2_pool.release()
        hht_pool.release()
        sp.release()

    return nc


_CACHED = {}


def _get_nc():
    if "nc" not in _CACHED:
        _CACHED["nc"] = build_nc()
    return _CACHED["nc"]


def prepare_in_maps(x, Wq, Wk, Wv, Wo, W1, b1, W2, b2, g1, be1, g2, be2):
    import ml_dtypes

    bf16 = ml_dtypes.bfloat16
    x = np.asarray(x, dtype=np.float32)
    Wq = np.asarray(Wq, np.float32).reshape(D, D)
    Wk = np.asarray(Wk, np.float32).reshape(D, D)
    Wv = np.asarray(Wv, np.float32).reshape(D, D)
    Wo = np.asarray(Wo, np.float32)
    W1 = np.asarray(W1, np.float32)
    b1 = np.asarray(b1, np.float32)
    W2 = np.asarray(W2, np.float32)
    b2 = np.asarray(b2, np.float32)
    g1 = np.asarray(g1, np.float32)
    be1 = np.asarray(be1, np.float32)
    g2 = np.asarray(g2, np.float32)
    be2 = np.asarray(be2, np.float32)

    w1p = W1 * g1[None, :]                      # fold LN1 gain
    b1p = b1 + W1 @ be1                         # fold LN1 bias
    from concourse import mybir as _mb

    f8 = _mb.dt.np(F8)
    # [d, ff] -> [j, k, i, ff] with d = j*256 + i*128 + k, interleaved
    # pairs packed in the free dim for DoubleRow matmuls.
    if FP8_W1:
        w1q = (W8SCALE * w1p.T).reshape(3, 2, P, DFF).transpose(0, 2, 1, 3)
        w1x = np.ascontiguousarray(w1q.reshape(3 * P, 2 * DFF)).astype(f8)
    else:
        w1x = np.ascontiguousarray(w1p.T).astype(bf16)
    if FP8_W2:
        w2q = (W8SCALE * W2.T).reshape(12, 2, P, D).transpose(0, 2, 1, 3)
        w2x = np.ascontiguousarray(w2q.reshape(12 * P, 2 * D)).astype(f8)
    else:
        w2x = np.ascontiguousarray(W2.T).astype(bf16)
    shared = {
        "wqt": np.ascontiguousarray(Wq.T).astype(bf16),
        "wkt": np.ascontiguousarray(Wk.T).astype(bf16),
        "wvt": np.ascontiguousarray(Wv.T).astype(bf16),
        "wot": np.ascontiguousarray(Wo.T).astype(bf16),
        "w1t": w1x,
        "w2t": w2x,
        "b1c": np.ascontiguousarray(
            (W8SCALE if FP8_W1 else 1.0) * b1p.reshape(FC, P).T
        ),
        "g1v": g1, "g2v": g2, "be2v": be2,
        "c2v": np.ascontiguousarray(be1 + b2),
    }
    in_maps = []
    for c in range(8):
        b, half = divmod(c, 2)
        if half == 0:
            xp = x[b]
        else:
            xp = np.concatenate([x[b, SQ:], x[b, :SQ]], axis=0)
        in_maps.append({
            "xt": np.ascontiguousarray(xp.T).astype(bf16),
            "xq": np.ascontiguousarray(xp[:SQ]).astype(bf16),
            **shared,
        })
    return in_maps


def kernel(
    x, Wq, Wk, Wv, Wo, W1, b1, W2, b2, g1, be1, g2, be2, _trace=False, **trace_kw
):
    in_maps = prepare_in_maps(
        x, Wq, Wk, Wv, Wo, W1, b1, W2, b2, g1, be1, g2, be2
    )
    nc = _get_nc()
    res = run_bass_kernel_spmd(
        nc, in_maps, core_ids=list(range(8)), trace=_trace, **trace_kw
    )
    out = np.empty((B, S, D), np.float32)
    for c in range(8):
        b, half = divmod(c, 2)
        out[b, half * SQ : (half + 1) * SQ] = res.results[c]["out"]
    if _trace:
        return out, res
    return out
